# revision 1
# baseline (speedup 1.0000x reference)
"""Chamfer loss kernel for Trainium2 (Bass/Tile), 8-core SPMD.

Per-core algorithm (2 batches/core, data-parallel over batch):
  -D[n,m] = 2x.y - ||x||^2 - ||y||^2 materialized tile-by-tile in PSUM
  via ONE bf16 matmul pass (K=68: 64 cross rows + 4 norm-lift rows,
  hi/lo split keeps the norms near-fp32; the 2e-2 harness gate leaves
  ~1000x margin at the measured ~9e-6 relative error).
  Per [128, 2048] unit (m-half-outer sweep, 32 n-tiles per half):
    ACT evacuates PSUM -> bf16 SBUF stage (0.92 ns/elem; every
    DVE_EVERY-th unit instead goes through DVE tensor_scalar+accum,
    which also yields its rowmax, to balance the two engines),
    DVE rowmax via tensor_scalar+accum in 4x_2p mode (0.27 ns/elem),
    DVE colmax via tensor_tensor bf16 running max in 2x_1p (0.54).
  Endgame: rowmax sums; colrun partition-max via GPSIMD
  partition_all_reduce for batch 0 (keeps PSUM free for the next
  batch's units) and bf16 PE transposes + DVE max-reduce for the last
  batch; per-lane partials [128, bpc] to DRAM.

Pipeline notes: the host hands each core two PURE-LAYOUT variants of
its input shard (d-major [64, N] for the matmul operands, n-major
[128, nt, 64] for the norms), so there is no on-device transpose or
DRAM bounce for the operands (only a tiny 4-row norm-lift scatter).
All DMAs are chunked to <=1.5us so no single transfer stalls the
critical path on the serial DMA-engine resource, and the next batch's
preproc chunks are emitted inline mid-sweep to fill idle gaps.
Host sums partials over lanes/batches/cores, negates, divides.

TimelineSim: 285104 ns (baseline 669158 ns; 2.35x).
"""

import os
from contextlib import ExitStack

import numpy as np

import concourse.bass as bass
import concourse.mybir as mybir
import concourse.bass_isa as bass_isa
from concourse import bacc
from concourse.tile import TileContext
from concourse.bass_utils import run_bass_kernel_spmd
from concourse.masks import make_identity

F32 = mybir.dt.float32
BF16 = mybir.dt.bfloat16
AX = mybir.AxisListType
OP = mybir.AluOpType
P = 128
BANK_F32 = 512          # fp32 elems per PSUM bank
UNIT_W = 2048           # unit width in m (4 banks)
NEG_INF = -3.0e38

B_FULL, N_FULL, M_FULL, D_FULL = 16, 4096, 4096, 64
NCORES = 8
BPC = B_FULL // NCORES  # batches per core

# Every DVE_EVERY-th unit is evacuated by DVE (tensor_scalar+accum from
# PSUM) instead of ACT. 0 disables. Tunes the ACT/DVE balance.
DVE_EVERY = int(os.environ.get("CHAMFER_DVE_EVERY", "24"))
SQ_DVE = bool(int(os.environ.get("CHAMFER_SQ_DVE", "1")))
PTT_SBUF = bool(int(os.environ.get("CHAMFER_PTT_SBUF", "0")))
SBUFS = int(os.environ.get("CHAMFER_SBUFS", "6"))
JBUFS = int(os.environ.get("CHAMFER_JBUFS", "2"))


def emit_chamfer(tc, pred_d, targ_d, pred_nm, targ_nm, out, bpc, n, m, d):
    nc = tc.nc
    nt = n // P
    unit_w = min(UNIT_W, m)
    nu = m // unit_w
    nb = unit_w // BANK_F32
    mt = m // P
    HT = nt // 2                # n-tiles per half

    ctx = ExitStack()
    const = ctx.enter_context(tc.tile_pool(name="const", bufs=1))
    bpool = ctx.enter_context(tc.tile_pool(name="batch", bufs=2))
    spool = ctx.enter_context(tc.tile_pool(name="stage", bufs=SBUFS))
    jpool = ctx.enter_context(tc.tile_pool(name="junk", bufs=JBUFS))
    cpool = ctx.enter_context(tc.tile_pool(name="colr", bufs=2))
    ppool = ctx.enter_context(tc.tile_pool(name="psum", bufs=2, space="PSUM"))
    dpool = ctx.enter_context(tc.tile_pool(name="dram", bufs=2, space="DRAM"))
    opool = ctx.enter_context(tc.tile_pool(name="outp", bufs=1))
    prpool = ctx.enter_context(tc.tile_pool(name="parp", bufs=1))

    identb = const.tile([P, P], BF16, tag="identb")
    make_identity(nc, identb[:])

    import ml_dtypes
    ones2 = np.ones((2, n), dtype=ml_dtypes.bfloat16)
    const_p1 = nc.inline_tensor(ones2, name="const_p1").ap()
    negones2 = -np.ones((2, m), dtype=ml_dtypes.bfloat16)
    const_m1 = nc.inline_tensor(negones2, name="const_m1").ap()

    totals = opool.tile([P, bpc], F32, tag="totals")

    # persistent per-batch operand/result tiles, rotated via tag bufs=2
    uA = {}
    vA = {}

    def preproc_half(side, b, half):
        """One side (x|y), one half of the points: build the [68, 2048]
        operand columns: bf16 cross rows (x carries the 2x scale) +
        norm-lift rows via a small DRAM scatter bounce."""
        dma = nc.sync if side == "x" else nc.scalar
        srcd = pred_d[b] if side == "x" else targ_d[b]
        srcn = pred_nm[b] if side == "x" else targ_nm[b]
        opmap = uA if side == "x" else vA
        if half == 0:
            opmap[b] = bpool.tile([d + 4, n], BF16, tag=f"op{side}",
                                  name=f"op{side}{b}")
        op = opmap[b]
        csl = slice(half * (n // 2), (half + 1) * (n // 2))

        # cross rows: load d-major fp32, cast (+scale) on ACT
        xd = bpool.tile([d, n // 2], F32, tag=f"{side}d{half}")
        for q in range(2):
            qs = slice(q * (n // 4), (q + 1) * (n // 4))
            dma.dma_start(xd[:, qs], srcd[:, csl][:, qs])
        if side == "x":
            nc.vector.tensor_scalar_mul(op[0:d, csl], xd[:], 2.0)
        else:
            nc.vector.tensor_copy(out=op[0:d, csl], in_=xd[:])

        # norms: load n-major fp32, square+reduce, hi/lo (+negate for y)
        ndma = dma
        xn = bpool.tile([P, HT, d], F32, tag=f"{side}n{half}")
        ndma.dma_start(xn[:], srcn[:, half * HT:(half + 1) * HT])
        sq = bpool.tile([P, 2, HT], F32, tag=f"sq{side}{half}")
        for c0 in range(0, HT, 8):
            tmp = bpool.tile([P, 8, d], F32, tag=f"sqt{side}{half}")
            if SQ_DVE:
                nc.vector.tensor_tensor(
                    tmp[:], xn[:, c0:c0 + 8], xn[:, c0:c0 + 8], OP.mult)
            else:
                nc.scalar.square(tmp[:], xn[:, c0:c0 + 8])
            nc.vector.tensor_reduce(
                sq[:, 0, c0:c0 + 8], tmp[:], axis=AX.X, op=OP.add)
        sqh = bpool.tile([P, 2, HT], BF16, tag=f"sqh{side}{half}")
        nc.vector.tensor_copy(out=sqh[:, 0], in_=sq[:, 0])
        nc.vector.tensor_tensor(sq[:, 1], sq[:, 0], sqh[:, 0], OP.subtract)
        if side == "y":
            nc.vector.tensor_scalar_mul(sqh[:, 0], sqh[:, 0], -1.0)
            nc.vector.tensor_scalar_mul(sq[:, 1], sq[:, 1], -1.0)
        nc.vector.tensor_copy(out=sqh[:, 1], in_=sq[:, 1])

        # scatter hi/lo rows through DRAM to become operand rows
        sr = dpool.tile([2, n], BF16, tag=f"sr{side}", name=f"sr{side}{b}") \
            if half == 0 else preproc_half.sr[side]
        preproc_half.sr[side] = sr
        with nc.allow_non_contiguous_dma(reason="sq-row scatter"):
            dma.dma_start(
                sr[0, csl].rearrange("(t p) -> p t", p=P), sqh[:, 0])
            dma.dma_start(
                sr[1, csl].rearrange("(t p) -> p t", p=P), sqh[:, 1])
        r0 = d if side == "x" else d + 2
        dma.dma_start(op[r0:r0 + 2, csl], sr[:, csl])
        c0 = d + 2 if side == "x" else d
        csrc = const_p1 if side == "x" else const_m1
        if half == 0:
            dma.dma_start(op[c0:c0 + 2, :], csrc)
    preproc_half.sr = {}

    def unit(b, i, h):
        u_op, v_op = uA[b], vA[b]
        nsl = slice(i * P, (i + 1) * P)
        base = h * unit_w
        pt = ppool.tile([P, unit_w], F32, tag="pt")
        for j in range(nb):
            bs = slice(j * BANK_F32, (j + 1) * BANK_F32)
            nc.tensor.matmul(
                pt[:, bs], u_op[:, nsl], v_op[:, base + j * BANK_F32:
                                              base + (j + 1) * BANK_F32],
                start=True, stop=True)
        k = h * nt + i
        dve_evac = DVE_EVERY > 0 and (k % DVE_EVERY == DVE_EVERY - 1)
        stage = spool.tile([P, unit_w], BF16, tag="stage")
        if dve_evac:
            nc.vector.tensor_scalar(
                out=stage[:], in0=pt[:], scalar1=NEG_INF, scalar2=None,
                op0=OP.max, op1=OP.max, accum_out=rm[h][:, i:i + 1])
        else:
            nc.scalar.copy(stage[:], pt[:])
            junk = jpool.tile([P, unit_w], BF16, tag="junk")
            nc.vector.tensor_scalar(
                out=junk[:], in0=stage[:], scalar1=NEG_INF, scalar2=None,
                op0=OP.max, op1=OP.max, accum_out=rm[h][:, i:i + 1])
        if i == 0:
            unit.stage0[h] = stage
        elif i == 1:
            nc.vector.tensor_tensor(
                colrun[h][:], stage[:], unit.stage0[h][:], OP.max)
        else:
            nc.vector.tensor_tensor(
                colrun[h][:], stage[:], colrun[h][:], OP.max)

    unit.stage0 = {}

    for b in range(bpc):
        if b == 0:
            preproc_half("x", 0, 0)
            preproc_half("y", 0, 0)

        colrun = [cpool.tile([P, unit_w], BF16, tag=f"colrun{h}",
                             name=f"colrun{h}_{b}")
                  for h in range(nu)]
        rm = [bpool.tile([P, nt], F32, tag=f"rm{h}", name=f"rm{h}_{b}")
              for h in range(nu)]

        nxt = b + 1 if b + 1 < bpc else None
        for h in range(nu):
            for i in range(nt):
                if b == 0 and h == 0:
                    if i == 8:
                        preproc_half("x", 0, 1)
                    elif i == 16:
                        preproc_half("y", 0, 1)
                if h == 1 and nxt is not None:
                    if i == 0:
                        preproc_half("x", nxt, 0)
                    elif i == 6:
                        preproc_half("y", nxt, 0)
                    elif i == 12:
                        preproc_half("x", nxt, 1)
                    elif i == 18:
                        preproc_half("y", nxt, 1)
                unit(b, i, h)

        # ---- endgame ----
        rmm = bpool.tile([P, nt], F32, tag="rmm")
        nc.vector.tensor_tensor(rmm[:], rm[0][:], rm[1][:], OP.max)
        rsum = bpool.tile([P, 1], F32, tag="rsum")
        nc.vector.tensor_reduce(rsum[:], rmm[:], axis=AX.X, op=OP.add)

        if b + 1 < bpc:
            # Pool partition-max endgame: keeps PSUM free for the next
            # batch's units (the transpose path holds a pt-ring slot).
            pr = prpool.tile([P, m], BF16, tag="pr", name=f"pr{b}")
            for h in range(nu):
                nc.gpsimd.partition_all_reduce(
                    pr[:, h * unit_w:(h + 1) * unit_w], colrun[h][:],
                    channels=P, reduce_op=bass_isa.ReduceOp.max)
            csum1 = bpool.tile([P, 1], F32, tag="csum1")
            nc.vector.tensor_reduce(
                csum1[0:1, :], pr[0:1, :], axis=AX.X, op=OP.add)
            nc.vector.tensor_copy(out=totals[:, b:b + 1], in_=rsum[:])
            nc.vector.tensor_tensor(
                totals[0:1, b:b + 1], csum1[0:1, :], totals[0:1, b:b + 1],
                OP.add)
        else:
            cm = bpool.tile([P, mt], F32, tag="cm")
            ptt = ppool.tile([P, m], BF16, tag="pt")
            for t in range(mt):
                h, tr = divmod(t, unit_w // P)
                nc.tensor.matmul(
                    ptt[:, t * P:(t + 1) * P],
                    colrun[h][:, tr * P:(tr + 1) * P], identb[:],
                    is_transpose=True,
                    start=(t % 8 == 0), stop=(t % 8 == 7))
            nc.vector.tensor_reduce(
                cm[:], ptt[:].rearrange("p (t q) -> p t q", q=P),
                axis=AX.X, op=OP.max)
            csum = bpool.tile([P, 1], F32, tag="csum")
            nc.vector.tensor_reduce(csum[:], cm[:], axis=AX.X, op=OP.add)
            nc.vector.tensor_tensor(
                totals[:, b:b + 1], rsum[:], csum[:], OP.add)

    nc.sync.dma_start(out[:], totals[:])
    ctx.close()


def build_program(bpc=BPC, n=N_FULL, m=M_FULL, d=D_FULL, debug=False):
    nc = bacc.Bacc(
        "TRN2", target_bir_lowering=False, debug=debug, enable_asserts=False)
    pred_d = nc.dram_tensor("pred_d", (bpc, d, n), F32, kind="ExternalInput").ap()
    targ_d = nc.dram_tensor("targ_d", (bpc, d, m), F32, kind="ExternalInput").ap()
    pred_nm = nc.dram_tensor(
        "pred_nm", (bpc, P, n // P, d), F32, kind="ExternalInput").ap()
    targ_nm = nc.dram_tensor(
        "targ_nm", (bpc, P, m // P, d), F32, kind="ExternalInput").ap()
    out = nc.dram_tensor("partials", (P, bpc), F32, kind="ExternalOutput").ap()
    with TileContext(nc, pool_alloc_mode="queue") as tc:
        emit_chamfer(tc, pred_d, targ_d, pred_nm, targ_nm, out, bpc, n, m, d)
    nc.compile()
    return nc


_NC_CACHE = {}


def _get_program():
    key = (BPC, N_FULL, M_FULL, D_FULL)
    if key not in _NC_CACHE:
        _NC_CACHE[key] = build_program(*key)
    return _NC_CACHE[key]


def kernel(pred_set, target_set):
    pred = np.ascontiguousarray(np.asarray(pred_set, dtype=np.float32))
    targ = np.ascontiguousarray(np.asarray(target_set, dtype=np.float32))
    assert pred.shape == (B_FULL, N_FULL, D_FULL), pred.shape
    assert targ.shape == (B_FULL, M_FULL, D_FULL), targ.shape

    nc = _get_program()
    in_maps = []
    for c in range(NCORES):
        ps = pred[c * BPC:(c + 1) * BPC]
        ts = targ[c * BPC:(c + 1) * BPC]
        in_maps.append({
            # pure layout transforms of this core's shard
            "pred_d": np.ascontiguousarray(ps.transpose(0, 2, 1)),
            "targ_d": np.ascontiguousarray(ts.transpose(0, 2, 1)),
            "pred_nm": np.ascontiguousarray(
                ps.reshape(BPC, N_FULL // P, P, D_FULL).transpose(0, 2, 1, 3)),
            "targ_nm": np.ascontiguousarray(
                ts.reshape(BPC, M_FULL // P, P, D_FULL).transpose(0, 2, 1, 3)),
        })
    trace = bool(int(os.environ.get("CHAMFER_TRACE", "0")))
    res = run_bass_kernel_spmd(
        nc, in_maps, core_ids=list(range(NCORES)), trace=trace)
    kernel.last_results = res
    total = 0.0
    for r in res.results:
        total += float(r["partials"].astype(np.float64).sum())
    val = -total / (float(N_FULL) * B_FULL)
    return np.float32(val)

